# revision 17
# baseline (speedup 1.0000x reference)
"""Balanced CE loss + accuracy on 8 Trainium2 NeuronCores (Bass/Tile).

Reference computation (N = 16777216 elements):
    loss = -sum(where(t==1, 1.6*log(p), 0.4*log(1-p))) / N
    acc  = mean(round(p) == t)

Strategy (data-parallel over N, no collectives needed):
  Shard N across 8 cores; per core stream [128, C] sub-chunks so the
  DMA pipe never idles.  Single-variable encoding y = 1 - |p - t| folds
  both classes into one value:
    t==1 -> y = p,  t==0 -> y = 1-p
  so the per-element log term is ln(y) with class weight w = 1.2*t+0.4,
  and "correct" (round(p)==t) is exactly y >= 0.5  <=>  ln(y) >= -ln2.
  y is produced bf16 by ONE fused custom-DVE op (1 - maxx(p-t, t-p)),
  which kills both the separate z pass and the |z| pass and halves all
  downstream SBUF traffic (the kernel is SBUF-port-bound, ~36B/elem):
    DVE : y = 1-|p-t| bf16         (custom op, fp32 internal)
          S1[s] = sum (t>=1)*q     (stt, fused accum -> sum_{t=1} ln p)
          m = (q >= -ln2) bf16     (plain tensor_scalar, fast mode)
    ACT : q = Ln(y) bf16           (fused accum -> S[s] = sum ln(y))
    PE  : ones^T @ m -> one accumulating PSUM bank -> correct count
  bf16 y/q cost ~0.4% relative per element -- random sign, so the 16M
  sums keep ~5 digits; counting against the bf16-rounded -ln2 shifts
  acc by ~2e-4 relative.  Both are ~100x under the 2e-2 gate.
  Sub-chunks ramp [512,1024,1536,2048*5,1536,1024,512] so the pipeline
  fills fast and the last dependent chain is short; the custom-y ops
  are emitted two chunks ahead of the Ln-dependent tail ops so the DVE
  queue never head-of-line blocks on ACT.
  Host folds the [128, 2*NCH+1] partials in f64:
    loss = -(0.4*S + 1.2*S1)/N,  acc = C/N.
"""

import sys

if "/opt/trn_rl_repo" not in sys.path:
    sys.path.insert(0, "/opt/trn_rl_repo")

import numpy as np

import concourse.bass as bass
import concourse.bacc as bacc
import concourse.tile as tile
from concourse import mybir
from concourse.bass_utils import run_bass_kernel_spmd
import concourse.dve_ops as dve_ops
from concourse.dve_ops import DveOp, OPS, CUSTOM_DVE_SPECS
from concourse.dve_spec import Spec, Src0, Src1, One, maxx, lower, _has_src1
from concourse.dve_uop import DveOpSpec

N_CORES = 8
N = 16777216
P = 128
SHARD = N // N_CORES          # 2097152 elements per core
COLS = SHARD // P             # 16384 columns per core
MMCOL = 512                   # matmul free-dim tile (one PSUM bank)

# chunk sizes: ramp up for fast pipeline fill, ramp down so the last
# y->q->{stt,mask} chain is short
SIZES = [512, 1024, 1536] + [2048] * 5 + [1536, 1024, 512]
assert sum(SIZES) == COLS
NCH = len(SIZES)

AF = mybir.ActivationFunctionType
OP = mybir.AluOpType
LN2 = 0.6931471805599453

_NC_CACHE = None


def _ref_y(in0, in1, c0, c1, c2):
    return 1.0 - np.abs(in0.astype(np.float32) - in1.astype(np.float32))


def _register_custom_op():
    """Register y = 1 - |p - t| as a runtime custom-DVE op."""
    name = "Y_FROM_PT_ANT"
    if name in dve_ops._SUB_OPCODE_FOR_NAME:
        return next(op for op in OPS if op.name == name)
    spec = Spec(body=One - maxx(Src0 - Src1, Src1 - Src0), reference=_ref_y)
    row = max(dve_ops._SUB_OPCODE_FOR_NAME.values()) + 1
    assert row < 0x20
    dve_ops._SUB_OPCODE_FOR_NAME[name] = row
    shas = {}
    for ver in ("v3", "v4"):
        s = DveOpSpec(name=name, opcode=row, uops=lower(spec, ver=ver),
                      rd1_en=_has_src1(spec))
        shas[ver] = s.sha(ver)
    op = DveOp(name, spec, subdim=False, uops_sha=shas)
    OPS.append(op)
    CUSTOM_DVE_SPECS[name] = spec
    return op


def build_bass():
    """Build the single-core Bass program (SPMD across 8 cores)."""
    global _NC_CACHE
    if _NC_CACHE is not None:
        return _NC_CACHE

    y_op = _register_custom_op()

    nc = bacc.Bacc("TRN2", target_bir_lowering=False, debug=False)

    p_in = nc.dram_tensor("p_in", [SHARD], mybir.dt.float32, kind="ExternalInput").ap()
    t_in = nc.dram_tensor("t_in", [SHARD], mybir.dt.int32, kind="ExternalInput").ap()
    # acc columns: [s] sum ln(y); [NCH+s] sum_{t=1} ln(p); [2*NCH] count
    acc = nc.dram_tensor("acc", [P, 2 * NCH + 1], mybir.dt.float32, kind="ExternalOutput").ap()

    n_mm = COLS // MMCOL

    with tile.TileContext(nc) as tc:
        with (
            tc.tile_pool(name="io", bufs=6) as io_pool,
            tc.tile_pool(name="yp", bufs=5) as y_pool,
            tc.tile_pool(name="qp", bufs=6) as q_pool,
            tc.tile_pool(name="mp", bufs=4) as m_pool,
            tc.tile_pool(name="psum", bufs=1, space=bass.MemorySpace.PSUM) as psum_pool,
            tc.tile_pool(name="misc", bufs=1) as misc_pool,
        ):
            ones = misc_pool.tile([P, P], mybir.dt.bfloat16, tag="ones")
            nc.gpsimd.memset(ones[:], 1.0)
            # warm the ACT Ln table under the DMA fill
            warm = misc_pool.tile([P, 1], mybir.dt.float32, tag="warm")
            nc.gpsimd.memset(warm[:], 0.5)
            nc.scalar.activation(warm[:], warm[:], AF.Ln)
            acc_a = misc_pool.tile([P, NCH], mybir.dt.float32, tag="acca")
            acc_v = misc_pool.tile([P, NCH + 1], mybir.dt.float32, tag="accv")
            junk_s = misc_pool.tile([P, max(SIZES)], mybir.dt.bfloat16, tag="js")
            ps = psum_pool.tile([P, MMCOL], mybir.dt.float32, tag="ps")

            mm = 0
            MX = max(SIZES)
            offs = [sum(SIZES[:i]) * P for i in range(NCH)]
            tiles = {}

            def issue_front(s):
                """DMA chunk s and compute y_s (software-pipelined ahead)."""
                sz = SIZES[s]
                p_f = io_pool.tile([P, MX], mybir.dt.float32, tag="p")
                t_f = io_pool.tile([P, MX], mybir.dt.int32, tag="t")
                y_f = y_pool.tile([P, MX], mybir.dt.bfloat16, tag="y")
                p_t, t_t, y_t = p_f[:, 0:sz], t_f[:, 0:sz], y_f[:, 0:sz]
                off = offs[s]
                nc.sync.dma_start(
                    p_t, p_in[off : off + sz * P].rearrange("(p f) -> p f", p=P)
                )
                nc.sync.dma_start(
                    t_t, t_in[off : off + sz * P].rearrange("(p f) -> p f", p=P)
                )
                # y = 1 - |p - t|  (one fused DVE op, bf16 out)
                nc.vector._custom_dve(y_op, out=y_t, in0=p_t, in1=t_t)
                tiles[s] = (t_t, y_t)

            def issue_back(s):
                """Ln + reductions for chunk s (runs Z_AHEAD behind)."""
                nonlocal mm
                sz = SIZES[s]
                t_t, y_t = tiles.pop(s)
                q_f = q_pool.tile([P, MX], mybir.dt.bfloat16, tag="q")
                m_f = m_pool.tile([P, MX], mybir.dt.bfloat16, tag="m")
                q_t, m_t = q_f[:, 0:sz], m_f[:, 0:sz]
                # q = ln(y); accum -> S[s]
                nc.scalar.activation(q_t, y_t, AF.Ln,
                                     accum_out=acc_a[:, s : s + 1])
                # S1[s] = sum_{t=1} q = sum_{t=1} ln(p)
                nc.vector.scalar_tensor_tensor(junk_s[:, 0:sz], t_t, 1, q_t,
                                               OP.is_ge, OP.mult,
                                               accum_out=acc_v[:, s : s + 1])
                # correct-count mask (bf16 in/out -> fast DVE), PE-reduced
                nc.vector.tensor_scalar(m_t, q_t, -LN2, None, OP.is_ge)
                for j in range(sz // MMCOL):
                    nc.tensor.matmul(
                        ps[:], ones[:], m_t[:, j * MMCOL : (j + 1) * MMCOL],
                        start=(mm == 0), stop=(mm == n_mm - 1),
                    )
                    mm += 1

            Z_AHEAD = 3
            for s in range(NCH + Z_AHEAD):
                if s < NCH:
                    issue_front(s)
                if s - Z_AHEAD >= 0:
                    issue_back(s - Z_AHEAD)

            # fold the PSUM count matrix (128 identical rows) into a column
            junk_f = misc_pool.tile([P, MMCOL], mybir.dt.float32, tag="jf")
            nc.vector.tensor_scalar(junk_f[:], ps[:], 1.0 / P, None, OP.mult,
                                    OP.add, accum_out=acc_v[:, NCH : NCH + 1])

            nc.sync.dma_start(acc[:, 0:NCH], acc_a[:])
            nc.sync.dma_start(acc[:, NCH : 2 * NCH + 1], acc_v[:])

    nc.finalize()
    _NC_CACHE = nc
    return nc


def make_in_maps(input, target):
    inp = np.ascontiguousarray(np.asarray(input, dtype=np.float32)).reshape(
        N_CORES, SHARD
    )
    tgt = np.ascontiguousarray(np.asarray(target, dtype=np.int32)).reshape(
        N_CORES, SHARD
    )
    return [{"p_in": inp[c], "t_in": tgt[c]} for c in range(N_CORES)]


def combine(results):
    """Host-side unshard: reduce the 8 cores' partial sums -> (loss, acc)."""
    S = S1 = C = 0.0
    for r in results:
        aa = np.asarray(r["acc"], dtype=np.float64)
        S += aa[:, 0:NCH].sum()
        S1 += aa[:, NCH : 2 * NCH].sum()
        C += aa[:, 2 * NCH].sum()
    loss = -(0.4 * S + 1.2 * S1) / N
    acc = C / N
    return np.float32(loss), np.float32(acc)


def run_on_hw(input, target, **spmd_kwargs):
    nc = build_bass()
    in_maps = make_in_maps(input, target)
    return run_bass_kernel_spmd(nc, in_maps, list(range(N_CORES)), **spmd_kwargs)


def kernel(input, target):
    br = run_on_hw(input, target)
    return combine(br.results)


# revision 18
# speedup vs baseline: 1.0022x; 1.0022x over previous
"""Balanced CE loss + accuracy on 8 Trainium2 NeuronCores (Bass/Tile).

Reference computation (N = 16777216 elements):
    loss = -sum(where(t==1, 1.6*log(p), 0.4*log(1-p))) / N
    acc  = mean(round(p) == t)

Strategy (data-parallel over N, no collectives needed):
  Shard N across 8 cores; per core stream [128, C] sub-chunks so the
  DMA pipe never idles.  Single-variable encoding y = 1 - |p - t| folds
  both classes into one value:
    t==1 -> y = p,  t==0 -> y = 1-p
  so the per-element log term is ln(y) with class weight w = 1.2*t+0.4,
  and "correct" (round(p)==t) is exactly y >= 0.5  <=>  ln(y) >= -ln2.
  y is produced bf16 by ONE fused custom-DVE op (1 - maxx(p-t, t-p)),
  which kills both the separate z pass and the |z| pass and halves all
  downstream SBUF traffic (the kernel is SBUF-port-bound, ~36B/elem):
    DVE : y = 1-|p-t| bf16         (custom op, fp32 internal)
          S1[s] = sum (t>=1)*q     (stt, fused accum -> sum_{t=1} ln p)
          m = (q >= -ln2) bf16     (plain tensor_scalar, fast mode)
    ACT : q = Ln(y) bf16           (fused accum -> S[s] = sum ln(y))
    PE  : ones^T @ m -> one accumulating PSUM bank -> correct count
  bf16 y/q cost ~0.4% relative per element -- random sign, so the 16M
  sums keep ~5 digits; counting against the bf16-rounded -ln2 shifts
  acc by ~2e-4 relative.  Both are ~100x under the 2e-2 gate.
  Sub-chunks ramp [512,1024,1536,2048*5,1536,1024,512] so the pipeline
  fills fast and the last dependent chain is short; the custom-y ops
  are emitted two chunks ahead of the Ln-dependent tail ops so the DVE
  queue never head-of-line blocks on ACT.
  Host folds the [128, 2*NCH+1] partials in f64:
    loss = -(0.4*S + 1.2*S1)/N,  acc = C/N.
"""

import sys

if "/opt/trn_rl_repo" not in sys.path:
    sys.path.insert(0, "/opt/trn_rl_repo")

import numpy as np

import concourse.bass as bass
import concourse.bacc as bacc
import concourse.tile as tile
from concourse import mybir
from concourse.bass_utils import run_bass_kernel_spmd
import concourse.dve_ops as dve_ops
from concourse.dve_ops import DveOp, OPS, CUSTOM_DVE_SPECS
from concourse.dve_spec import Spec, Src0, Src1, One, maxx, lower, _has_src1
from concourse.dve_uop import DveOpSpec

N_CORES = 8
N = 16777216
P = 128
SHARD = N // N_CORES          # 2097152 elements per core
COLS = SHARD // P             # 16384 columns per core
MMCOL = 512                   # matmul free-dim tile (one PSUM bank)

# chunk sizes: ramp up for fast pipeline fill, ramp down so the last
# y->q->{stt,mask} chain is short
SIZES = [512, 1024, 1536] + [2048] * 5 + [1536, 1024, 512]
assert sum(SIZES) == COLS
NCH = len(SIZES)

AF = mybir.ActivationFunctionType
OP = mybir.AluOpType
LN2 = 0.6931471805599453

_NC_CACHE = None


def _ref_y(in0, in1, c0, c1, c2):
    return 1.0 - np.abs(in0.astype(np.float32) - in1.astype(np.float32))


def _register_custom_op():
    """Register y = 1 - |p - t| as a runtime custom-DVE op."""
    name = "Y_FROM_PT_ANT"
    if name in dve_ops._SUB_OPCODE_FOR_NAME:
        return next(op for op in OPS if op.name == name)
    spec = Spec(body=One - maxx(Src0 - Src1, Src1 - Src0), reference=_ref_y)
    row = max(dve_ops._SUB_OPCODE_FOR_NAME.values()) + 1
    assert row < 0x20
    dve_ops._SUB_OPCODE_FOR_NAME[name] = row
    shas = {}
    for ver in ("v3", "v4"):
        s = DveOpSpec(name=name, opcode=row, uops=lower(spec, ver=ver),
                      rd1_en=_has_src1(spec))
        shas[ver] = s.sha(ver)
    op = DveOp(name, spec, subdim=False, uops_sha=shas)
    OPS.append(op)
    CUSTOM_DVE_SPECS[name] = spec
    return op


def build_bass():
    """Build the single-core Bass program (SPMD across 8 cores)."""
    global _NC_CACHE
    if _NC_CACHE is not None:
        return _NC_CACHE

    y_op = _register_custom_op()

    nc = bacc.Bacc("TRN2", target_bir_lowering=False, debug=False)

    p_in = nc.dram_tensor("p_in", [SHARD], mybir.dt.float32, kind="ExternalInput").ap()
    t_in = nc.dram_tensor("t_in", [SHARD], mybir.dt.int32, kind="ExternalInput").ap()
    # acc columns: [s] sum ln(y); [NCH+s] sum_{t=1} ln(p); [2*NCH] count
    acc = nc.dram_tensor("acc", [P, 2 * NCH + 1], mybir.dt.float32, kind="ExternalOutput").ap()

    n_mm = COLS // MMCOL

    with tile.TileContext(nc) as tc:
        with (
            tc.tile_pool(name="io", bufs=6) as io_pool,
            tc.tile_pool(name="yp", bufs=5) as y_pool,
            tc.tile_pool(name="qp", bufs=6) as q_pool,
            tc.tile_pool(name="mp", bufs=4) as m_pool,
            tc.tile_pool(name="psum", bufs=1, space=bass.MemorySpace.PSUM) as psum_pool,
            tc.tile_pool(name="misc", bufs=1) as misc_pool,
        ):
            ones = misc_pool.tile([P, P], mybir.dt.bfloat16, tag="ones")
            nc.gpsimd.memset(ones[:], 1.0)
            # warm the ACT Ln table under the DMA fill
            warm = misc_pool.tile([P, 1], mybir.dt.float32, tag="warm")
            nc.gpsimd.memset(warm[:], 0.5)
            nc.scalar.activation(warm[:], warm[:], AF.Ln)
            acc_a = misc_pool.tile([P, NCH], mybir.dt.float32, tag="acca")
            acc_v = misc_pool.tile([P, NCH + 1], mybir.dt.float32, tag="accv")
            junk_s = misc_pool.tile([P, max(SIZES)], mybir.dt.bfloat16, tag="js")
            ps = psum_pool.tile([P, MMCOL], mybir.dt.float32, tag="ps")

            mm = 0
            MX = max(SIZES)
            offs = [sum(SIZES[:i]) * P for i in range(NCH)]
            tiles = {}

            def issue_front(s):
                """DMA chunk s and compute y_s (software-pipelined ahead)."""
                sz = SIZES[s]
                p_f = io_pool.tile([P, MX], mybir.dt.float32, tag="p")
                t_f = io_pool.tile([P, MX], mybir.dt.int32, tag="t")
                y_f = y_pool.tile([P, MX], mybir.dt.bfloat16, tag="y")
                p_t, t_t, y_t = p_f[:, 0:sz], t_f[:, 0:sz], y_f[:, 0:sz]
                off = offs[s]
                nc.sync.dma_start(
                    p_t, p_in[off : off + sz * P].rearrange("(p f) -> p f", p=P)
                )
                nc.sync.dma_start(
                    t_t, t_in[off : off + sz * P].rearrange("(p f) -> p f", p=P)
                )
                # y = 1 - |p - t|  (one fused DVE op, bf16 out)
                nc.vector._custom_dve(y_op, out=y_t, in0=p_t, in1=t_t)
                tiles[s] = (t_t, y_t)

            def issue_back(s):
                """Ln + reductions for chunk s (runs Z_AHEAD behind)."""
                nonlocal mm
                sz = SIZES[s]
                t_t, y_t = tiles.pop(s)
                q_f = q_pool.tile([P, MX], mybir.dt.bfloat16, tag="q")
                m_f = m_pool.tile([P, MX], mybir.dt.bfloat16, tag="m")
                q_t, m_t = q_f[:, 0:sz], m_f[:, 0:sz]
                # q = ln(y); accum -> S[s]
                nc.scalar.activation(q_t, y_t, AF.Ln,
                                     accum_out=acc_a[:, s : s + 1])
                # S1[s] = sum_{t=1} q = sum_{t=1} ln(p)
                nc.vector.scalar_tensor_tensor(junk_s[:, 0:sz], t_t, 1, q_t,
                                               OP.is_ge, OP.mult,
                                               accum_out=acc_v[:, s : s + 1])
                # correct-count mask (bf16 in/out -> fast DVE), PE-reduced
                nc.vector.tensor_scalar(m_t, q_t, -LN2, None, OP.is_ge)
                for j in range(sz // MMCOL):
                    nc.tensor.matmul(
                        ps[:], ones[:], m_t[:, j * MMCOL : (j + 1) * MMCOL],
                        start=(mm == 0), stop=(mm == n_mm - 1),
                    )
                    mm += 1

            Z_AHEAD = NCH
            for s in range(NCH + Z_AHEAD):
                if s < NCH:
                    issue_front(s)
                if s - Z_AHEAD >= 0:
                    issue_back(s - Z_AHEAD)

            # fold the PSUM count matrix (128 identical rows) into a column
            junk_f = misc_pool.tile([P, MMCOL], mybir.dt.float32, tag="jf")
            nc.vector.tensor_scalar(junk_f[:], ps[:], 1.0 / P, None, OP.mult,
                                    OP.add, accum_out=acc_v[:, NCH : NCH + 1])

            nc.sync.dma_start(acc[:, 0:NCH], acc_a[:])
            nc.sync.dma_start(acc[:, NCH : 2 * NCH + 1], acc_v[:])

    nc.finalize()
    _NC_CACHE = nc
    return nc


def make_in_maps(input, target):
    inp = np.ascontiguousarray(np.asarray(input, dtype=np.float32)).reshape(
        N_CORES, SHARD
    )
    tgt = np.ascontiguousarray(np.asarray(target, dtype=np.int32)).reshape(
        N_CORES, SHARD
    )
    return [{"p_in": inp[c], "t_in": tgt[c]} for c in range(N_CORES)]


def combine(results):
    """Host-side unshard: reduce the 8 cores' partial sums -> (loss, acc)."""
    S = S1 = C = 0.0
    for r in results:
        aa = np.asarray(r["acc"], dtype=np.float64)
        S += aa[:, 0:NCH].sum()
        S1 += aa[:, NCH : 2 * NCH].sum()
        C += aa[:, 2 * NCH].sum()
    loss = -(0.4 * S + 1.2 * S1) / N
    acc = C / N
    return np.float32(loss), np.float32(acc)


def run_on_hw(input, target, **spmd_kwargs):
    nc = build_bass()
    in_maps = make_in_maps(input, target)
    return run_bass_kernel_spmd(nc, in_maps, list(range(N_CORES)), **spmd_kwargs)


def kernel(input, target):
    br = run_on_hw(input, target)
    return combine(br.results)


# revision 20
# speedup vs baseline: 1.0270x; 1.0247x over previous
"""Balanced CE loss + accuracy on 8 Trainium2 NeuronCores (Bass/Tile).

Reference computation (N = 16777216 elements):
    loss = -sum(where(t==1, 1.6*log(p), 0.4*log(1-p))) / N
    acc  = mean(round(p) == t)

Strategy (data-parallel over N, no collectives needed):
  Shard N across 8 cores; per core stream [128, C] sub-chunks so the
  DMA pipe never idles.  Single-variable encoding y = 1 - |p - t| folds
  both classes into one value:
    t==1 -> y = p,  t==0 -> y = 1-p
  so the per-element log term is ln(y) with class weight w = 1.2*t+0.4,
  and "correct" (round(p)==t) is exactly y >= 0.5  <=>  ln(y) >= -ln2.
  y is produced bf16 by ONE fused custom-DVE op (1 - maxx(p-t, t-p)),
  which kills both the separate z pass and the |z| pass and halves all
  downstream SBUF traffic (the kernel is SBUF-port-bound, ~36B/elem):
    DVE : y = 1-|p-t| bf16         (custom op, fp32 internal)
          S1[s] = sum (t>=1)*q     (stt, fused accum -> sum_{t=1} ln p)
          m = (q >= -ln2) bf16     (plain tensor_scalar, fast mode)
    ACT : q = Ln(y) bf16           (fused accum -> S[s] = sum ln(y))
    PE  : ones^T @ m -> one accumulating PSUM bank -> correct count
  bf16 y/q cost ~0.4% relative per element -- random sign, so the 16M
  sums keep ~5 digits; counting against the bf16-rounded -ln2 shifts
  acc by ~2e-4 relative.  Both are ~100x under the 2e-2 gate.
  Sub-chunks ramp [512,1024,1536,2048*5,1536,1024,512] so the pipeline
  fills fast and the last dependent chain is short; the custom-y ops
  are emitted two chunks ahead of the Ln-dependent tail ops so the DVE
  queue never head-of-line blocks on ACT.
  Host folds the [128, 2*NCH+1] partials in f64:
    loss = -(0.4*S + 1.2*S1)/N,  acc = C/N.
"""

import sys

if "/opt/trn_rl_repo" not in sys.path:
    sys.path.insert(0, "/opt/trn_rl_repo")

import numpy as np

import concourse.bass as bass
import concourse.bacc as bacc
import concourse.tile as tile
from concourse import mybir
from concourse.bass_utils import run_bass_kernel_spmd
import concourse.hw_specs as hw_specs
import concourse.dve_ops as dve_ops
from concourse.dve_ops import DveOp, OPS, CUSTOM_DVE_SPECS
from concourse.dve_spec import Spec, Src0, Src1, One, maxx, lower, _has_src1
from concourse.dve_uop import DveOpSpec

N_CORES = 8
N = 16777216
P = 128
SHARD = N // N_CORES          # 2097152 elements per core
COLS = SHARD // P             # 16384 columns per core
MMCOL = 512                   # matmul free-dim tile (one PSUM bank)

# chunk sizes: ramp up for fast pipeline fill, ramp down so the last
# y->q->{stt,mask} chain is short
SIZES = [512, 1024, 1536] + [2048] * 5 + [1536, 1024, 512]
assert sum(SIZES) == COLS
NCH = len(SIZES)

AF = mybir.ActivationFunctionType
OP = mybir.AluOpType
LN2 = 0.6931471805599453

_NC_CACHE = None

# The Tile list-scheduler orders engine streams from a CoreSim pass using
# TRN2Spec timings.  Its default DMA model (0.83 derate) believes input
# chunks land SLOWER than the DVE drains them, so it schedules each
# chunk's stt before the next chunk's y-op and the real machine then
# serializes Ln->stt->y->Ln cross-engine.  Believing a slightly faster
# DMA flips the order to y-first, which is what the real machine needs.
hw_specs.TRN2Spec.DMA_CYCLE = 1e9 / (400e9 / 128) / 1.15


def _ref_y(in0, in1, c0, c1, c2):
    return 1.0 - np.abs(in0.astype(np.float32) - in1.astype(np.float32))


def _register_custom_op():
    """Register y = 1 - |p - t| as a runtime custom-DVE op."""
    name = "Y_FROM_PT_ANT"
    if name in dve_ops._SUB_OPCODE_FOR_NAME:
        return next(op for op in OPS if op.name == name)
    spec = Spec(body=One - maxx(Src0 - Src1, Src1 - Src0), reference=_ref_y)
    row = max(dve_ops._SUB_OPCODE_FOR_NAME.values()) + 1
    assert row < 0x20
    dve_ops._SUB_OPCODE_FOR_NAME[name] = row
    shas = {}
    for ver in ("v3", "v4"):
        s = DveOpSpec(name=name, opcode=row, uops=lower(spec, ver=ver),
                      rd1_en=_has_src1(spec))
        shas[ver] = s.sha(ver)
    op = DveOp(name, spec, subdim=False, uops_sha=shas)
    OPS.append(op)
    CUSTOM_DVE_SPECS[name] = spec
    return op


def build_bass():
    """Build the single-core Bass program (SPMD across 8 cores)."""
    global _NC_CACHE
    if _NC_CACHE is not None:
        return _NC_CACHE

    y_op = _register_custom_op()

    nc = bacc.Bacc("TRN2", target_bir_lowering=False, debug=False)

    p_in = nc.dram_tensor("p_in", [SHARD], mybir.dt.float32, kind="ExternalInput").ap()
    t_in = nc.dram_tensor("t_in", [SHARD], mybir.dt.int32, kind="ExternalInput").ap()
    # acc cols: [s] sum ln(y); [NCH+s] sum sign(ln y + ln2); [2NCH+s] sum_{t=1} ln p
    acc = nc.dram_tensor("acc", [P, 3 * NCH], mybir.dt.float32, kind="ExternalOutput").ap()

    with tile.TileContext(nc) as tc:
        with (
            tc.tile_pool(name="io", bufs=6) as io_pool,
            tc.tile_pool(name="yp", bufs=5) as y_pool,
            tc.tile_pool(name="qp", bufs=6) as q_pool,
            tc.tile_pool(name="misc", bufs=1) as misc_pool,
        ):
            # warm the ACT Ln table under the DMA fill; ln2 bias as a
            # tracked tile (a float const would need a pre-context const AP)
            warm = misc_pool.tile([P, 1], mybir.dt.float32, tag="warm")
            ln2c = misc_pool.tile([P, 1], mybir.dt.float32, tag="ln2c")
            nc.gpsimd.memset(warm[:], 0.5)
            nc.gpsimd.memset(ln2c[:], LN2)
            nc.scalar.activation(warm[:], warm[:], AF.Ln)
            acc_a = misc_pool.tile([P, 2 * NCH], mybir.dt.float32, tag="acca")
            acc_v = misc_pool.tile([P, NCH], mybir.dt.float32, tag="accv")
            junk_s = misc_pool.tile([P, max(SIZES)], mybir.dt.bfloat16, tag="js")
            junk_g = misc_pool.tile([P, max(SIZES)], mybir.dt.bfloat16, tag="jg")

            MX = max(SIZES)
            offs = [sum(SIZES[:i]) * P for i in range(NCH)]
            tiles = {}

            def issue_front(s):
                """DMA chunk s and compute y_s (software-pipelined ahead)."""
                sz = SIZES[s]
                p_f = io_pool.tile([P, MX], mybir.dt.float32, tag="p")
                t_f = io_pool.tile([P, MX], mybir.dt.int32, tag="t")
                y_f = y_pool.tile([P, MX], mybir.dt.bfloat16, tag="y")
                p_t, t_t, y_t = p_f[:, 0:sz], t_f[:, 0:sz], y_f[:, 0:sz]
                off = offs[s]
                nc.sync.dma_start(
                    p_t, p_in[off : off + sz * P].rearrange("(p f) -> p f", p=P)
                )
                nc.sync.dma_start(
                    t_t, t_in[off : off + sz * P].rearrange("(p f) -> p f", p=P)
                )
                # y = 1 - |p - t|  (one fused DVE op, bf16 out)
                nc.vector._custom_dve(y_op, out=y_t, in0=p_t, in1=t_t)
                tiles[s] = (t_t, y_t)

            def issue_back(s):
                """Ln + reductions for chunk s."""
                sz = SIZES[s]
                t_t, y_t = tiles.pop(s)
                q_f = q_pool.tile([P, MX], mybir.dt.bfloat16, tag="q")
                q_t = q_f[:, 0:sz]
                # q = ln(y); accum -> S[s]
                nc.scalar.activation(q_t, y_t, AF.Ln,
                                     accum_out=acc_a[:, s : s + 1])
                # count: sign(q + ln2) = +-1, accum -> G[s]; correct = (G+n)/2
                nc.scalar.activation(junk_g[:, 0:sz], q_t, AF.Sign, bias=ln2c[:, 0:1],
                                     accum_out=acc_a[:, NCH + s : NCH + s + 1])
                # S1[s] = sum_{t=1} q = sum_{t=1} ln(p)
                nc.vector.scalar_tensor_tensor(junk_s[:, 0:sz], t_t, 1, q_t,
                                               OP.is_ge, OP.mult,
                                               accum_out=acc_v[:, s : s + 1])

            Z_AHEAD = NCH
            for s in range(NCH + Z_AHEAD):
                if s < NCH:
                    issue_front(s)
                if s - Z_AHEAD >= 0:
                    issue_back(s - Z_AHEAD)

            nc.sync.dma_start(acc[:, 0 : 2 * NCH], acc_a[:])
            nc.sync.dma_start(acc[:, 2 * NCH : 3 * NCH], acc_v[:])

    nc.finalize()
    _NC_CACHE = nc
    return nc


def make_in_maps(input, target):
    inp = np.ascontiguousarray(np.asarray(input, dtype=np.float32)).reshape(
        N_CORES, SHARD
    )
    tgt = np.ascontiguousarray(np.asarray(target, dtype=np.int32)).reshape(
        N_CORES, SHARD
    )
    return [{"p_in": inp[c], "t_in": tgt[c]} for c in range(N_CORES)]


def combine(results):
    """Host-side unshard: reduce the 8 cores' partial sums -> (loss, acc)."""
    S = S1 = G = 0.0
    for r in results:
        aa = np.asarray(r["acc"], dtype=np.float64)
        S += aa[:, 0:NCH].sum()
        G += aa[:, NCH : 2 * NCH].sum()
        S1 += aa[:, 2 * NCH : 3 * NCH].sum()
    loss = -(0.4 * S + 1.2 * S1) / N
    acc = (G + N) / 2.0 / N
    return np.float32(loss), np.float32(acc)


def run_on_hw(input, target, **spmd_kwargs):
    nc = build_bass()
    in_maps = make_in_maps(input, target)
    return run_bass_kernel_spmd(nc, in_maps, list(range(N_CORES)), **spmd_kwargs)


def kernel(input, target):
    br = run_on_hw(input, target)
    return combine(br.results)


# revision 21
# speedup vs baseline: 1.1155x; 1.0861x over previous
"""Balanced CE loss + accuracy on 8 Trainium2 NeuronCores (Bass/Tile).

Reference computation (N = 16777216 elements):
    loss = -sum(where(t==1, 1.6*log(p), 0.4*log(1-p))) / N
    acc  = mean(round(p) == t)

Strategy (data-parallel over N, no collectives needed):
  Shard N across 8 cores; per core stream [128, C] sub-chunks so the
  DMA pipe never idles.  Single-variable encoding y = 1 - |p - t| folds
  both classes into one value:
    t==1 -> y = p,  t==0 -> y = 1-p
  so the per-element log term is ln(y) with class weight w = 1.2*t+0.4,
  and "correct" (round(p)==t) is exactly y >= 0.5  <=>  ln(y) >= -ln2.
  y is produced bf16 by ONE fused custom-DVE op (1 - maxx(p-t, t-p)),
  which kills both the separate z pass and the |z| pass and halves all
  downstream SBUF traffic (the kernel is SBUF-port-bound, ~36B/elem):
    DVE : y = 1-|p-t| bf16         (custom op, fp32 internal)
          S1[s] = sum (t>=1)*q     (stt, fused accum -> sum_{t=1} ln p)
          m = (q >= -ln2) bf16     (plain tensor_scalar, fast mode)
    ACT : q = Ln(y) bf16           (fused accum -> S[s] = sum ln(y))
    PE  : ones^T @ m -> one accumulating PSUM bank -> correct count
  bf16 y/q cost ~0.4% relative per element -- random sign, so the 16M
  sums keep ~5 digits; counting against the bf16-rounded -ln2 shifts
  acc by ~2e-4 relative.  Both are ~100x under the 2e-2 gate.
  Sub-chunks ramp [512,1024,1536,2048*5,1536,1024,512] so the pipeline
  fills fast and the last dependent chain is short; the custom-y ops
  are emitted two chunks ahead of the Ln-dependent tail ops so the DVE
  queue never head-of-line blocks on ACT.
  Host folds the [128, 2*NCH+1] partials in f64:
    loss = -(0.4*S + 1.2*S1)/N,  acc = C/N.
"""

import sys

if "/opt/trn_rl_repo" not in sys.path:
    sys.path.insert(0, "/opt/trn_rl_repo")

import numpy as np

import concourse.bass as bass
import concourse.bacc as bacc
import concourse.tile as tile
from concourse import mybir
from concourse.bass_utils import run_bass_kernel_spmd
import concourse.hw_specs as hw_specs
import concourse.dve_ops as dve_ops
from concourse.dve_ops import DveOp, OPS, CUSTOM_DVE_SPECS
from concourse.dve_spec import Spec, Src0, Src1, One, maxx, lower, _has_src1
from concourse.dve_uop import DveOpSpec

N_CORES = 8
N = 16777216
P = 128
SHARD = N // N_CORES          # 2097152 elements per core
COLS = SHARD // P             # 16384 columns per core
MMCOL = 512                   # matmul free-dim tile (one PSUM bank)

# chunk sizes: ramp up for fast pipeline fill, ramp down so the last
# y->q->{stt,mask} chain is short
SIZES = [512, 1024, 1536] + [2048] * 5 + [1536, 1024, 512]
assert sum(SIZES) == COLS
NCH = len(SIZES)

AF = mybir.ActivationFunctionType
OP = mybir.AluOpType
LN2 = 0.6931471805599453

_NC_CACHE = None

# The Tile list-scheduler orders engine streams from a CoreSim pass using
# TRN2Spec timings.  Its default DMA model (0.83 derate) believes input
# chunks land SLOWER than the DVE drains them, so it schedules each
# chunk's stt before the next chunk's y-op and the real machine then
# serializes Ln->stt->y->Ln cross-engine.  Believing a slightly faster
# DMA flips the order to y-first, which is what the real machine needs.
hw_specs.TRN2Spec.DMA_CYCLE = 1e9 / (400e9 / 128) / 1.15


def _ref_y(in0, in1, c0, c1, c2):
    return 1.0 - np.abs(in0.astype(np.float32) - in1.astype(np.float32))


def _register_custom_op():
    """Register y = 1 - |p - t| as a runtime custom-DVE op."""
    name = "Y_FROM_PT_ANT"
    if name in dve_ops._SUB_OPCODE_FOR_NAME:
        return next(op for op in OPS if op.name == name)
    spec = Spec(body=One - maxx(Src0 - Src1, Src1 - Src0), reference=_ref_y)
    row = max(dve_ops._SUB_OPCODE_FOR_NAME.values()) + 1
    assert row < 0x20
    dve_ops._SUB_OPCODE_FOR_NAME[name] = row
    shas = {}
    for ver in ("v3", "v4"):
        s = DveOpSpec(name=name, opcode=row, uops=lower(spec, ver=ver),
                      rd1_en=_has_src1(spec))
        shas[ver] = s.sha(ver)
    op = DveOp(name, spec, subdim=False, uops_sha=shas)
    OPS.append(op)
    CUSTOM_DVE_SPECS[name] = spec
    return op


def build_bass():
    """Build the single-core Bass program (SPMD across 8 cores)."""
    global _NC_CACHE
    if _NC_CACHE is not None:
        return _NC_CACHE

    y_op = _register_custom_op()

    nc = bacc.Bacc("TRN2", target_bir_lowering=False, debug=False)

    p_in = nc.dram_tensor("p_in", [SHARD], mybir.dt.float32, kind="ExternalInput").ap()
    t_in = nc.dram_tensor("t_in", [SHARD], mybir.dt.int32, kind="ExternalInput").ap()
    # acc cols: [s] sum ln(y); [NCH+s] sum sign(ln y + ln2); [2NCH+s] sum_{t=1} ln p
    acc = nc.dram_tensor("acc", [P, 3 * NCH], mybir.dt.float32, kind="ExternalOutput").ap()

    with tile.TileContext(nc) as tc:
        with (
            tc.tile_pool(name="io", bufs=6) as io_pool,
            tc.tile_pool(name="yp", bufs=5) as y_pool,
            tc.tile_pool(name="qp", bufs=6) as q_pool,
            tc.tile_pool(name="misc", bufs=1) as misc_pool,
            tc.tile_pool(name="psj", bufs=1, space=bass.MemorySpace.PSUM) as psum_pool,
        ):
            # warm the ACT Ln table under the DMA fill; ln2 bias as a
            # tracked tile (a float const would need a pre-context const AP)
            warm = misc_pool.tile([P, 1], mybir.dt.float32, tag="warm")
            ln2c = misc_pool.tile([P, 1], mybir.dt.float32, tag="ln2c")
            nc.gpsimd.memset(warm[:], 0.5)
            nc.gpsimd.memset(ln2c[:], LN2)
            nc.scalar.activation(warm[:], warm[:], AF.Ln)
            acc_a = misc_pool.tile([P, 2 * NCH], mybir.dt.float32, tag="acca")
            acc_v = misc_pool.tile([P, NCH], mybir.dt.float32, tag="accv")
            junk_s = psum_pool.tile([P, max(SIZES)], mybir.dt.float32, tag="js")
            junk_g = psum_pool.tile([P, max(SIZES)], mybir.dt.float32, tag="jg")

            MX = max(SIZES)
            offs = [sum(SIZES[:i]) * P for i in range(NCH)]
            tiles = {}

            def issue_front(s):
                """DMA chunk s and compute y_s (software-pipelined ahead)."""
                sz = SIZES[s]
                p_f = io_pool.tile([P, MX], mybir.dt.float32, tag="p")
                t_f = io_pool.tile([P, MX], mybir.dt.int32, tag="t")
                y_f = y_pool.tile([P, MX], mybir.dt.bfloat16, tag="y")
                p_t, t_t, y_t = p_f[:, 0:sz], t_f[:, 0:sz], y_f[:, 0:sz]
                off = offs[s]
                nc.sync.dma_start(
                    p_t, p_in[off : off + sz * P].rearrange("(p f) -> p f", p=P)
                )
                nc.sync.dma_start(
                    t_t, t_in[off : off + sz * P].rearrange("(p f) -> p f", p=P)
                )
                # y = 1 - |p - t|  (one fused DVE op, bf16 out)
                nc.vector._custom_dve(y_op, out=y_t, in0=p_t, in1=t_t)
                tiles[s] = (t_t, y_t)

            def issue_back(s):
                """Ln + reductions for chunk s."""
                sz = SIZES[s]
                t_t, y_t = tiles.pop(s)
                q_f = q_pool.tile([P, MX], mybir.dt.bfloat16, tag="q")
                q_t = q_f[:, 0:sz]
                # q = ln(y); accum -> S[s]
                nc.scalar.activation(q_t, y_t, AF.Ln,
                                     accum_out=acc_a[:, s : s + 1])
                # count: sign(q + ln2) = +-1, accum -> G[s]; correct = (G+n)/2
                nc.scalar.activation(junk_g[:, 0:sz], q_t, AF.Sign, bias=ln2c[:, 0:1],
                                     accum_out=acc_a[:, NCH + s : NCH + s + 1])
                # S1[s] = sum_{t=1} q = sum_{t=1} ln(p)
                nc.vector.scalar_tensor_tensor(junk_s[:, 0:sz], t_t, 1, q_t,
                                               OP.is_ge, OP.mult,
                                               accum_out=acc_v[:, s : s + 1])

            Z_AHEAD = NCH
            for s in range(NCH + Z_AHEAD):
                if s < NCH:
                    issue_front(s)
                if s - Z_AHEAD >= 0:
                    issue_back(s - Z_AHEAD)

            nc.sync.dma_start(acc[:, 0 : 2 * NCH], acc_a[:])
            nc.sync.dma_start(acc[:, 2 * NCH : 3 * NCH], acc_v[:])

    nc.finalize()
    _NC_CACHE = nc
    return nc


def make_in_maps(input, target):
    inp = np.ascontiguousarray(np.asarray(input, dtype=np.float32)).reshape(
        N_CORES, SHARD
    )
    tgt = np.ascontiguousarray(np.asarray(target, dtype=np.int32)).reshape(
        N_CORES, SHARD
    )
    return [{"p_in": inp[c], "t_in": tgt[c]} for c in range(N_CORES)]


def combine(results):
    """Host-side unshard: reduce the 8 cores' partial sums -> (loss, acc)."""
    S = S1 = G = 0.0
    for r in results:
        aa = np.asarray(r["acc"], dtype=np.float64)
        S += aa[:, 0:NCH].sum()
        G += aa[:, NCH : 2 * NCH].sum()
        S1 += aa[:, 2 * NCH : 3 * NCH].sum()
    loss = -(0.4 * S + 1.2 * S1) / N
    acc = (G + N) / 2.0 / N
    return np.float32(loss), np.float32(acc)


def run_on_hw(input, target, **spmd_kwargs):
    nc = build_bass()
    in_maps = make_in_maps(input, target)
    return run_bass_kernel_spmd(nc, in_maps, list(range(N_CORES)), **spmd_kwargs)


def kernel(input, target):
    br = run_on_hw(input, target)
    return combine(br.results)
